# revision 1
# baseline (speedup 1.0000x reference)
"""Trainium2 8-core Bass kernel for nn_BasicGcn (3-layer GCN, N=50000,
E=600000, D=128).

Strategy (sharding_hint: shard nodes + incident edges, replicate weights,
exchange boundary features between layers):
  - Nodes are sharded contiguously across the 8 NeuronCores (6250 each);
    the feature table is replicated per-core in HBM as fp16, split into two
    globally-concatenated segments A/B so gather indices fit int16.
  - Host preprocessing folds the full symmetric GCN normalization
    (deg^-1/2 incl. self-loops) into per-128-message one-hot "S" matrices.
  - Per layer each core, for groups of 128-target blocks: dma_gather the
    incident-edge source rows (messages, fp16), aggregate with S-chunk
    matmuls accumulated in PSUM (agg^T = sum_k M_k^T S_k), transform with
    the replicated 128x128 weight (lhsT=agg^T so the result lands
    [targets, dout] with no transpose), add bias via a K=1 ones-matmul,
    apply the activation (exact ELU / erf-GELU LUT / PReLU), and DMA the
    core's slice out; AllGathers (split per segment, partially overlapped
    with compute) rebuild the replicated table between layers.
  - fp16 storage with fp32 PSUM accumulation keeps |err|_max / |out|_max
    at ~1e-3 vs the fp32 reference.
"""
import numpy as np

import concourse.bacc as bacc
import concourse.mybir as mybir
import concourse.tile as tile
from concourse.bass_utils import run_bass_kernel_spmd

P = 128
D = 128
NCORES = 8
N_NODES = 50000

FP16 = mybir.dt.float16
FP32 = mybir.dt.float32
I16 = mybir.dt.int16


def _make_structure(n_nodes, group_size=5):
    N = n_nodes
    T = N // NCORES
    NBLK = (T + P - 1) // P
    ablk = (NBLK + 1) // 2
    arows = min(ablk * P, T)
    brows = T - arows
    groups = []
    for start in range(0, ablk, group_size):
        groups.append(list(range(start, min(start + group_size, ablk))))
    na_groups = len(groups)
    for start in range(ablk, NBLK, group_size):
        groups.append(list(range(start, min(start + group_size, NBLK))))
    return dict(N=N, T=T, NBLK=NBLK, ablk=ablk, arows=arows, brows=brows,
                groups=groups, na_groups=na_groups)


def _preprocess(edge_index, edge_weights, st):
    N, T, NBLK = st["N"], st["T"], st["NBLK"]
    arows, brows = st["arows"], st["brows"]
    groups = st["groups"]
    NG = len(groups)

    row = np.asarray(edge_index[0], dtype=np.int64)
    col = np.asarray(edge_index[1], dtype=np.int64)
    w = np.asarray(edge_weights, dtype=np.float64)

    deg = np.bincount(col, weights=w, minlength=N) + 1.0
    dinv = 1.0 / np.sqrt(deg)

    src = np.concatenate([row, np.arange(N, dtype=np.int64)])
    tgt = np.concatenate([col, np.arange(N, dtype=np.int64)])
    nrm = np.concatenate([dinv[row] * w * dinv[col], dinv * dinv]).astype(np.float32)

    core = tgt // T
    tloc = tgt - core * T
    blk = tloc // P
    slot_t = tloc % P

    s_core = src // T
    s_off = src - s_core * T
    sseg = (s_off >= arows).astype(np.int64)
    lidx = np.where(sseg == 0, s_core * arows + s_off,
                    s_core * brows + (s_off - arows))

    blk2grp = np.zeros(NBLK, dtype=np.int64)
    for gi, bs in enumerate(groups):
        blk2grp[bs] = gi
    grp = blk2grp[blk]

    keyb = ((core * NG + grp) * 2 + sseg) * NBLK + blk
    cntb = np.bincount(keyb, minlength=NCORES * NG * 2 * NBLK).reshape(
        NCORES, NG, 2, NBLK)
    ub = cntb.max(axis=0)

    reg_slots = ub.sum(axis=2)
    reg_chunks = -(-reg_slots // P)
    reg_base = np.zeros((NG, 2), dtype=np.int64)
    flat = (reg_chunks * P).reshape(-1)
    reg_base.reshape(-1)[1:] = np.cumsum(flat)[:-1]
    NSLOT = int(flat.sum())
    sub_base = np.zeros((NG, 2, NBLK), dtype=np.int64)
    for g in range(NG):
        for s in range(2):
            off = reg_base[g, s]
            for b in groups[g]:
                sub_base[g, s, b] = off
                off += ub[g, s, b]

    sched = [[] for _ in range(NBLK)]
    scol_of = {}
    g_scol_base = []
    scol = 0
    for g in range(NG):
        g_scol_base.append(scol)
        for s in range(2):
            for k in range(int(reg_chunks[g, s])):
                k_in_grp = k + (reg_chunks[g, 0] if s == 1 else 0)
                lo = reg_base[g, s] + k * P
                hi = lo + P
                for b in groups[g]:
                    b0 = sub_base[g, s, b]
                    b1 = b0 + ub[g, s, b]
                    if b0 < hi and b1 > lo and ub[g, s, b] > 0:
                        sched[b].append((int(k_in_grp), int(scol)))
                        scol_of[(g, s, k, b)] = scol
                        scol += 1
    STOT = scol

    idx_arr = np.zeros((NCORES, P, NSLOT // 16), dtype=np.int16)
    S_arr = np.zeros((NCORES, P, STOT * P), dtype=np.float16)

    order = np.lexsort((blk, sseg, grp, core))
    keyb_sorted = keyb[order]
    firsts = np.r_[0, np.nonzero(np.diff(keyb_sorted))[0] + 1]
    grp_start = np.zeros_like(keyb_sorted)
    grp_start[firsts] = np.arange(len(keyb_sorted))[firsts]
    np.maximum.accumulate(grp_start, out=grp_start)
    rank = np.arange(len(keyb_sorted)) - grp_start

    g_o = grp[order]
    s_o = sseg[order]
    b_o = blk[order]
    gslot = sub_base[g_o, s_o, b_o] + rank
    c_o = core[order]
    idx_arr[c_o, gslot % 16, gslot // 16] = lidx[order].astype(np.int16)
    k_reg = (gslot - reg_base[g_o, s_o]) // P
    lut = np.full((NG, 2, int(reg_chunks.max()), NBLK), -1, dtype=np.int64)
    for (g, s, k, b), v in scol_of.items():
        lut[g, s, k, b] = v
    sc_o = lut[g_o, s_o, k_reg, b_o]
    S_arr[c_o, gslot % P, sc_o * P + slot_t[order]] = nrm[order].astype(np.float16)

    # index pattern is read per-Q7-core: replicate over all 8 groups of 16
    idx_arr = np.ascontiguousarray(np.tile(idx_arr[:, :16, :], (1, 8, 1)))

    meta = dict(reg_chunks=reg_chunks, reg_base=reg_base, sched=sched,
                g_scol_base=g_scol_base, NSLOT=NSLOT, STOT=STOT)
    return idx_arr, S_arr, meta


def _build_kernel(st, meta, prelu_slope):
    T, NBLK, ablk = st["T"], st["NBLK"], st["ablk"]
    arows, brows = st["arows"], st["brows"]
    groups, na_groups = st["groups"], st["na_groups"]
    NG = len(groups)
    reg_chunks = meta["reg_chunks"]
    reg_base = meta["reg_base"]
    sched = meta["sched"]
    g_scol_base = meta["g_scol_base"]
    NSLOT, STOT = meta["NSLOT"], meta["STOT"]
    AG_ROWS = arows * NCORES
    BG_ROWS = brows * NCORES
    KG_MAX = int((reg_chunks[:, 0] + reg_chunks[:, 1]).max())
    SG_MAX = max(
        (g_scol_base[g + 1] if g + 1 < NG else STOT) - g_scol_base[g]
        for g in range(NG))

    nc = bacc.Bacc("TRN2", target_bir_lowering=False, debug=False,
                   num_devices=NCORES)
    xa = nc.dram_tensor("xa", [AG_ROWS, D], FP16, kind="ExternalInput")
    xb = nc.dram_tensor("xb", [BG_ROWS, D], FP16, kind="ExternalInput")
    s_d = nc.dram_tensor("s", [P, STOT * P], FP16, kind="ExternalInput")
    idx_d = nc.dram_tensor("idx", [P, NSLOT // 16], I16, kind="ExternalInput")
    w_d = nc.dram_tensor("wts", [P, 3 * P], FP16, kind="ExternalInput")
    b_d = nc.dram_tensor("bias", [1, 3 * P], FP32, kind="ExternalInput")
    y = nc.dram_tensor("y", [T, D], FP32, kind="ExternalOutput")

    with tile.TileContext(nc) as tc:
        with (
            tc.tile_pool(name="dram", bufs=2, space="DRAM") as dram,
            tc.tile_pool(name="const", bufs=1) as const,
            tc.tile_pool(name="s_sb", bufs=2) as s_pool,
            tc.tile_pool(name="m_sb", bufs=2) as m_pool,
            tc.tile_pool(name="h_sb", bufs=3) as h_pool,
            tc.tile_pool(name="ps_a", bufs=3, space="PSUM") as ps_a,
            tc.tile_pool(name="ps_b", bufs=2, space="PSUM") as ps_b,
        ):
            idx_t = const.tile([P, NSLOT // 16], I16)
            w_t = const.tile([P, 3 * P], FP16)
            b_t = const.tile([1, 3 * P], FP32)
            ones_t = const.tile([1, P], FP32)
            nc.sync.dma_start(out=idx_t[:], in_=idx_d[:])
            nc.sync.dma_start(out=w_t[:], in_=w_d[:])
            nc.sync.dma_start(out=b_t[:], in_=b_d[:])
            nc.vector.memset(ones_t[:], 1.0)

            tblsA, tblsB, bncA, bncB = [], [], [], []
            for l in range(2):
                tblsA.append(dram.tile([AG_ROWS, D], FP16, tag="tblA",
                                       name=f"tblA{l}"))
                tblsB.append(dram.tile([BG_ROWS, D], FP16, tag="tblB",
                                       name=f"tblB{l}"))
                bncA.append(dram.tile([arows, D], FP16, tag="bncA",
                                      name=f"bncA{l}"))
                bncB.append(dram.tile([brows, D], FP16, tag="bncB",
                                      name=f"bncB{l}"))

            for l in range(3):
                srcA = xa if l == 0 else tblsA[l - 1]
                srcB = xb if l == 0 else tblsB[l - 1]
                for g in range(NG):
                    kA = int(reg_chunks[g, 0])
                    kB = int(reg_chunks[g, 1])
                    kT = kA + kB
                    scol0 = g_scol_base[g]
                    scol1 = g_scol_base[g + 1] if g + 1 < NG else STOT
                    ns = scol1 - scol0

                    s_t = s_pool.tile([P, SG_MAX * P], FP16, tag="s")
                    nc.sync.dma_start(
                        out=s_t[:, :ns * P],
                        in_=s_d[:, scol0 * P:scol1 * P])

                    m_t = m_pool.tile([P, KG_MAX, P], FP16, tag="m")
                    baseA = int(reg_base[g, 0])
                    baseB = int(reg_base[g, 1])
                    if kA:
                        nc.gpsimd.dma_gather(
                            m_t[:, :kA, :], srcA[:, :],
                            idx_t[:, baseA // 16: baseA // 16 + kA * 8],
                            kA * P, kA * P, D, single_packet=False)
                    if kB:
                        nc.gpsimd.dma_gather(
                            m_t[:, kA:kT, :], srcB[:, :],
                            idx_t[:, baseB // 16: baseB // 16 + kB * 8],
                            kB * P, kB * P, D, single_packet=False)

                    for b in groups[g]:
                        ent = sched[b]
                        agg_ps = ps_a.tile([P, P], FP32, tag="agg")
                        for j, (k, scol) in enumerate(ent):
                            nc.tensor.matmul(
                                agg_ps[:],
                                lhsT=m_t[:, k, :],
                                rhs=s_t[:, (scol - scol0) * P:
                                        (scol - scol0 + 1) * P],
                                start=(j == 0), stop=(j == len(ent) - 1))

                        aggT = h_pool.tile([P, P], FP16, tag="aggT")
                        nc.scalar.copy(out=aggT[:], in_=agg_ps[:])

                        h_ps = ps_b.tile([P, P], FP32, tag="h")
                        nc.tensor.matmul(
                            h_ps[:], lhsT=aggT[:],
                            rhs=w_t[:, l * P:(l + 1) * P],
                            start=True, stop=False)
                        nc.tensor.matmul(
                            h_ps[:], lhsT=ones_t[:1, :],
                            rhs=b_t[:1, l * P:(l + 1) * P],
                            start=False, stop=True)

                        out_dt = FP16 if l < 2 else FP32
                        o_t = h_pool.tile([P, P], out_dt, tag=f"o{min(l, 1)}")
                        if l == 0:  # exact ELU = max(x,0)+exp(min(x,0))-1
                            mn = h_pool.tile([P, P], FP32, tag="mn")
                            mx = h_pool.tile([P, P], FP32, tag="mx")
                            ex = h_pool.tile([P, P], FP32, tag="ex")
                            nc.vector.tensor_scalar_min(mn[:], h_ps[:], 0.0)
                            nc.vector.tensor_scalar_max(mx[:], h_ps[:], 0.0)
                            nc.scalar.activation(
                                ex[:], mn[:], mybir.ActivationFunctionType.Exp)
                            sm = h_pool.tile([P, P], FP32, tag="sm")
                            nc.vector.tensor_tensor(
                                out=sm[:], in0=ex[:], in1=mx[:],
                                op=mybir.AluOpType.add)
                            nc.vector.tensor_scalar_add(o_t[:], sm[:], -1.0)
                        elif l == 1:  # erf-GELU via ACT LUT
                            nc.scalar.activation(
                                o_t[:], h_ps[:],
                                mybir.ActivationFunctionType.Gelu)
                        else:  # PReLU = max(x,0) + slope*min(x,0)
                            mn = h_pool.tile([P, P], FP32, tag="mn")
                            mx = h_pool.tile([P, P], FP32, tag="mx")
                            nc.vector.tensor_scalar(
                                mn[:], h_ps[:], 0.0, float(prelu_slope),
                                mybir.AluOpType.min, mybir.AluOpType.mult)
                            nc.vector.tensor_scalar_max(mx[:], h_ps[:], 0.0)
                            nc.vector.tensor_tensor(
                                out=o_t[:], in0=mn[:], in1=mx[:],
                                op=mybir.AluOpType.add)

                        rb = min(P, T - b * P)
                        if l == 2:
                            nc.sync.dma_start(
                                out=y[b * P:b * P + rb, :], in_=o_t[:rb, :])
                        elif b < ablk:
                            nc.sync.dma_start(
                                out=bncA[l][b * P:b * P + rb, :],
                                in_=o_t[:rb, :])
                        else:
                            r0 = (b - ablk) * P
                            nc.sync.dma_start(
                                out=bncB[l][r0:r0 + rb, :], in_=o_t[:rb, :])

                    if l < 2 and g == na_groups - 1:
                        nc.gpsimd.collective_compute(
                            "AllGather", mybir.AluOpType.bypass,
                            replica_groups=[list(range(NCORES))],
                            ins=[bncA[l].opt()], outs=[tblsA[l].opt()])
                if l < 2:
                    nc.gpsimd.collective_compute(
                        "AllGather", mybir.AluOpType.bypass,
                        replica_groups=[list(range(NCORES))],
                        ins=[bncB[l].opt()], outs=[tblsB[l].opt()])

    nc.compile()
    return nc


_CACHE = {}


def kernel(x, edge_index, edge_weights, W1, b1, W2, b2, W3, b3, prelu_w):
    """Full (unsharded) inputs in, full [50000, 128] fp32 output out."""
    st = _make_structure(N_NODES, group_size=5)
    idx_arr, S_arr, meta = _preprocess(edge_index, edge_weights, st)

    key = (meta["NSLOT"], meta["STOT"], float(prelu_w))
    if key not in _CACHE:
        _CACHE[key] = _build_kernel(st, meta, float(prelu_w))
    nc = _CACHE[key]

    T, arows = st["T"], st["arows"]
    x16 = np.asarray(x, dtype=np.float16).reshape(NCORES, T, D)
    xa = np.ascontiguousarray(x16[:, :arows, :].reshape(-1, D))
    xb = np.ascontiguousarray(x16[:, arows:, :].reshape(-1, D))
    wts = np.ascontiguousarray(np.concatenate(
        [np.asarray(W, dtype=np.float16) for W in (W1, W2, W3)], axis=1))
    bias = np.ascontiguousarray(np.concatenate(
        [np.asarray(b, dtype=np.float32) for b in (b1, b2, b3)])[None, :])
    in_maps = [
        {"xa": xa, "xb": xb, "s": np.ascontiguousarray(S_arr[c]),
         "idx": np.ascontiguousarray(idx_arr[c]), "wts": wts, "bias": bias}
        for c in range(NCORES)
    ]

    res = run_bass_kernel_spmd(nc, in_maps, core_ids=list(range(NCORES)))
    out = np.concatenate([res.results[c]["y"] for c in range(NCORES)], axis=0)
    return out.astype(np.float32)


# revision 2
# speedup vs baseline: 10.6855x; 10.6855x over previous
"""Trainium2 8-core Bass kernel for nn_BasicGcn (3-layer GCN, N=50000,
E=600000, D=128).

Strategy (sharding_hint: shard nodes + incident edges, replicate weights,
exchange boundary features between layers):
  - Nodes are sharded contiguously across the 8 NeuronCores (6250 each);
    the feature table is replicated per-core in HBM as fp16, split into two
    globally-concatenated segments A/B so gather indices fit int16.
  - Host preprocessing folds the full symmetric GCN normalization
    (deg^-1/2 incl. self-loops) into per-128-message one-hot "S" matrices.
  - Per layer each core, for groups of 128-target blocks: dma_gather the
    incident-edge source rows (messages, fp16), aggregate with S-chunk
    matmuls accumulated in PSUM (agg^T = sum_k M_k^T S_k), transform with
    the replicated 128x128 weight (lhsT=agg^T so the result lands
    [targets, dout] with no transpose), add bias via a K=1 ones-matmul,
    apply the activation (exact ELU / erf-GELU LUT / PReLU), and DMA the
    core's slice out; AllGathers (split per segment, partially overlapped
    with compute) rebuild the replicated table between layers.
  - S is a pure one-hot matrix stored fp8 (values {0,1} exact); the edge
    norm is applied to the gathered messages by a per-chunk DVE prescale
    from a small fp32 norm table, halving the S stream.
  - fp16 storage with fp32 PSUM accumulation keeps |err|_max / |out|_max
    at ~7e-4 vs the fp32 reference.
"""
import numpy as np

import concourse.bacc as bacc
import concourse.mybir as mybir
import concourse.tile as tile
from concourse.bass_utils import run_bass_kernel_spmd

P = 128
D = 128
NCORES = 8
N_NODES = 50000

FP16 = mybir.dt.float16
FP32 = mybir.dt.float32
FP8 = mybir.dt.float8e4
I16 = mybir.dt.int16


def _make_structure(n_nodes, group_size=5):
    N = n_nodes
    T = N // NCORES
    NBLK = (T + P - 1) // P
    ablk = (NBLK + 1) // 2
    arows = min(ablk * P, T)
    brows = T - arows
    groups = []
    for start in range(0, ablk, group_size):
        groups.append(list(range(start, min(start + group_size, ablk))))
    na_groups = len(groups)
    for start in range(ablk, NBLK, group_size):
        groups.append(list(range(start, min(start + group_size, NBLK))))
    return dict(N=N, T=T, NBLK=NBLK, ablk=ablk, arows=arows, brows=brows,
                groups=groups, na_groups=na_groups)


def _preprocess(edge_index, edge_weights, st):
    N, T, NBLK = st["N"], st["T"], st["NBLK"]
    arows, brows = st["arows"], st["brows"]
    groups = st["groups"]
    NG = len(groups)

    row = np.asarray(edge_index[0], dtype=np.int64)
    col = np.asarray(edge_index[1], dtype=np.int64)
    w = np.asarray(edge_weights, dtype=np.float64)

    deg = np.bincount(col, weights=w, minlength=N) + 1.0
    dinv = 1.0 / np.sqrt(deg)

    src = np.concatenate([row, np.arange(N, dtype=np.int64)])
    tgt = np.concatenate([col, np.arange(N, dtype=np.int64)])
    nrm = np.concatenate([dinv[row] * w * dinv[col], dinv * dinv]).astype(np.float32)

    core = tgt // T
    tloc = tgt - core * T
    blk = tloc // P
    slot_t = tloc % P

    s_core = src // T
    s_off = src - s_core * T
    sseg = (s_off >= arows).astype(np.int64)
    lidx = np.where(sseg == 0, s_core * arows + s_off,
                    s_core * brows + (s_off - arows))

    blk2grp = np.zeros(NBLK, dtype=np.int64)
    for gi, bs in enumerate(groups):
        blk2grp[bs] = gi
    grp = blk2grp[blk]

    keyb = ((core * NG + grp) * 2 + sseg) * NBLK + blk
    cntb = np.bincount(keyb, minlength=NCORES * NG * 2 * NBLK).reshape(
        NCORES, NG, 2, NBLK)
    ub = cntb.max(axis=0)

    reg_slots = ub.sum(axis=2)
    reg_chunks = -(-reg_slots // P)
    reg_base = np.zeros((NG, 2), dtype=np.int64)
    flat = (reg_chunks * P).reshape(-1)
    reg_base.reshape(-1)[1:] = np.cumsum(flat)[:-1]
    NSLOT = int(flat.sum())
    sub_base = np.zeros((NG, 2, NBLK), dtype=np.int64)
    for g in range(NG):
        for s in range(2):
            off = reg_base[g, s]
            for b in groups[g]:
                sub_base[g, s, b] = off
                off += ub[g, s, b]

    sched = [[] for _ in range(NBLK)]
    scol_of = {}
    g_scol_base = []
    scol = 0
    for g in range(NG):
        g_scol_base.append(scol)
        for s in range(2):
            for k in range(int(reg_chunks[g, s])):
                k_in_grp = k + (reg_chunks[g, 0] if s == 1 else 0)
                lo = reg_base[g, s] + k * P
                hi = lo + P
                for b in groups[g]:
                    b0 = sub_base[g, s, b]
                    b1 = b0 + ub[g, s, b]
                    if b0 < hi and b1 > lo and ub[g, s, b] > 0:
                        sched[b].append((int(k_in_grp), int(scol)))
                        scol_of[(g, s, k, b)] = scol
                        scol += 1
    STOT = scol

    import ml_dtypes
    idx_arr = np.zeros((NCORES, P, NSLOT // 16), dtype=np.int16)
    S_arr = np.zeros((NCORES, P, STOT * P), dtype=ml_dtypes.float8_e4m3fn)
    nrm_arr = np.zeros((NCORES, P, NSLOT // P), dtype=np.float32)

    order = np.lexsort((blk, sseg, grp, core))
    keyb_sorted = keyb[order]
    firsts = np.r_[0, np.nonzero(np.diff(keyb_sorted))[0] + 1]
    grp_start = np.zeros_like(keyb_sorted)
    grp_start[firsts] = np.arange(len(keyb_sorted))[firsts]
    np.maximum.accumulate(grp_start, out=grp_start)
    rank = np.arange(len(keyb_sorted)) - grp_start

    g_o = grp[order]
    s_o = sseg[order]
    b_o = blk[order]
    gslot = sub_base[g_o, s_o, b_o] + rank
    c_o = core[order]
    idx_arr[c_o, gslot % 16, gslot // 16] = lidx[order].astype(np.int16)
    k_reg = (gslot - reg_base[g_o, s_o]) // P
    lut = np.full((NG, 2, int(reg_chunks.max()), NBLK), -1, dtype=np.int64)
    for (g, s, k, b), v in scol_of.items():
        lut[g, s, k, b] = v
    sc_o = lut[g_o, s_o, k_reg, b_o]
    S_arr[c_o, gslot % P, sc_o * P + slot_t[order]] = 1.0
    nrm_arr[c_o, gslot % P, gslot // P] = nrm[order]

    # index pattern is read per-Q7-core: replicate over all 8 groups of 16
    idx_arr = np.ascontiguousarray(np.tile(idx_arr[:, :16, :], (1, 8, 1)))

    meta = dict(reg_chunks=reg_chunks, reg_base=reg_base, sched=sched,
                g_scol_base=g_scol_base, NSLOT=NSLOT, STOT=STOT)
    return idx_arr, S_arr, nrm_arr, meta


def _build_kernel(st, meta, prelu_slope):
    T, NBLK, ablk = st["T"], st["NBLK"], st["ablk"]
    arows, brows = st["arows"], st["brows"]
    groups, na_groups = st["groups"], st["na_groups"]
    NG = len(groups)
    reg_chunks = meta["reg_chunks"]
    reg_base = meta["reg_base"]
    sched = meta["sched"]
    g_scol_base = meta["g_scol_base"]
    NSLOT, STOT = meta["NSLOT"], meta["STOT"]
    AG_ROWS = arows * NCORES
    BG_ROWS = brows * NCORES
    KG_MAX = int((reg_chunks[:, 0] + reg_chunks[:, 1]).max())
    SG_MAX = max(
        (g_scol_base[g + 1] if g + 1 < NG else STOT) - g_scol_base[g]
        for g in range(NG))

    nc = bacc.Bacc("TRN2", target_bir_lowering=False, debug=False,
                   num_devices=NCORES)
    xa = nc.dram_tensor("xa", [AG_ROWS, D], FP16, kind="ExternalInput")
    xb = nc.dram_tensor("xb", [BG_ROWS, D], FP16, kind="ExternalInput")
    s_d = nc.dram_tensor("s", [P, STOT * P], FP8, kind="ExternalInput")
    n_d = nc.dram_tensor("nrm", [P, NSLOT // P], FP32, kind="ExternalInput")
    idx_d = nc.dram_tensor("idx", [P, NSLOT // 16], I16, kind="ExternalInput")
    w_d = nc.dram_tensor("wts", [P, 3 * P], FP16, kind="ExternalInput")
    b_d = nc.dram_tensor("bias", [1, 3 * P], FP32, kind="ExternalInput")
    y = nc.dram_tensor("y", [T, D], FP32, kind="ExternalOutput")

    with tile.TileContext(nc) as tc:
        with (
            tc.tile_pool(name="dram", bufs=2, space="DRAM") as dram,
            tc.tile_pool(name="const", bufs=1) as const,
            tc.tile_pool(name="s_sb", bufs=3) as s_pool,
            tc.tile_pool(name="m_sb", bufs=3) as m_pool,
            tc.tile_pool(name="h_sb", bufs=3) as h_pool,
            tc.tile_pool(name="ps_a", bufs=3, space="PSUM") as ps_a,
            tc.tile_pool(name="ps_b", bufs=2, space="PSUM") as ps_b,
        ):
            idx_t = const.tile([P, NSLOT // 16], I16)
            n_t = const.tile([P, NSLOT // P], FP32)
            nc.sync.dma_start(out=n_t[:], in_=n_d[:])
            w_t = const.tile([P, 3 * P], FP16)
            b_t = const.tile([1, 3 * P], FP32)
            ones_t = const.tile([1, P], FP32)
            nc.sync.dma_start(out=idx_t[:], in_=idx_d[:])
            nc.sync.dma_start(out=w_t[:], in_=w_d[:])
            nc.sync.dma_start(out=b_t[:], in_=b_d[:])
            nc.vector.memset(ones_t[:], 1.0)

            tblsA, tblsB, bncA, bncB = [], [], [], []
            for l in range(2):
                tblsA.append(dram.tile([AG_ROWS, D], FP16, tag="tblA",
                                       name=f"tblA{l}"))
                tblsB.append(dram.tile([BG_ROWS, D], FP16, tag="tblB",
                                       name=f"tblB{l}"))
                bncA.append(dram.tile([arows, D], FP16, tag="bncA",
                                      name=f"bncA{l}"))
                bncB.append(dram.tile([brows, D], FP16, tag="bncB",
                                      name=f"bncB{l}"))

            for l in range(3):
                srcA = xa if l == 0 else tblsA[l - 1]
                srcB = xb if l == 0 else tblsB[l - 1]
                for g in range(NG):
                    kA = int(reg_chunks[g, 0])
                    kB = int(reg_chunks[g, 1])
                    kT = kA + kB
                    scol0 = g_scol_base[g]
                    scol1 = g_scol_base[g + 1] if g + 1 < NG else STOT
                    ns = scol1 - scol0

                    s_t = s_pool.tile([P, SG_MAX * P], FP8, tag="s")
                    nc.sync.dma_start(
                        out=s_t[:, :ns * P],
                        in_=s_d[:, scol0 * P:scol1 * P])

                    m_t = m_pool.tile([P, KG_MAX, P], FP16, tag="m")
                    baseA = int(reg_base[g, 0])
                    baseB = int(reg_base[g, 1])
                    if kA:
                        nc.gpsimd.dma_gather(
                            m_t[:, :kA, :], srcA[:, :],
                            idx_t[:, baseA // 16: baseA // 16 + kA * 8],
                            kA * P, kA * P, D, single_packet=False)
                    if kB:
                        nc.gpsimd.dma_gather(
                            m_t[:, kA:kT, :], srcB[:, :],
                            idx_t[:, baseB // 16: baseB // 16 + kB * 8],
                            kB * P, kB * P, D, single_packet=False)

                    for k in range(kT):
                        kg = (baseA // P + k if k < kA
                              else baseB // P + (k - kA))
                        nc.vector.tensor_scalar_mul(
                            m_t[:, k, :], m_t[:, k, :], n_t[:, kg:kg + 1])

                    for b in groups[g]:
                        ent = sched[b]
                        agg_ps = ps_a.tile([P, P], FP32, tag="agg")
                        for j, (k, scol) in enumerate(ent):
                            nc.tensor.matmul(
                                agg_ps[:],
                                lhsT=m_t[:, k, :],
                                rhs=s_t[:, (scol - scol0) * P:
                                        (scol - scol0 + 1) * P],
                                start=(j == 0), stop=(j == len(ent) - 1))

                        aggT = h_pool.tile([P, P], FP16, tag="aggT")
                        nc.scalar.copy(out=aggT[:], in_=agg_ps[:])

                        h_ps = ps_b.tile([P, P], FP32, tag="h")
                        nc.tensor.matmul(
                            h_ps[:], lhsT=aggT[:],
                            rhs=w_t[:, l * P:(l + 1) * P],
                            start=True, stop=False)
                        nc.tensor.matmul(
                            h_ps[:], lhsT=ones_t[:1, :],
                            rhs=b_t[:1, l * P:(l + 1) * P],
                            start=False, stop=True)

                        out_dt = FP16 if l < 2 else FP32
                        o_t = h_pool.tile([P, P], out_dt, tag=f"o{min(l, 1)}")
                        if l == 0:  # exact ELU = max(x,0)+exp(min(x,0))-1
                            mn = h_pool.tile([P, P], FP32, tag="mn")
                            mx = h_pool.tile([P, P], FP32, tag="mx")
                            ex = h_pool.tile([P, P], FP32, tag="ex")
                            nc.vector.tensor_scalar_min(mn[:], h_ps[:], 0.0)
                            nc.vector.tensor_scalar_max(mx[:], h_ps[:], 0.0)
                            nc.scalar.activation(
                                ex[:], mn[:], mybir.ActivationFunctionType.Exp)
                            sm = h_pool.tile([P, P], FP32, tag="sm")
                            nc.vector.tensor_tensor(
                                out=sm[:], in0=ex[:], in1=mx[:],
                                op=mybir.AluOpType.add)
                            nc.vector.tensor_scalar_add(o_t[:], sm[:], -1.0)
                        elif l == 1:  # erf-GELU via ACT LUT
                            nc.scalar.activation(
                                o_t[:], h_ps[:],
                                mybir.ActivationFunctionType.Gelu)
                        else:  # PReLU = max(x,0) + slope*min(x,0)
                            mn = h_pool.tile([P, P], FP32, tag="mn")
                            mx = h_pool.tile([P, P], FP32, tag="mx")
                            nc.vector.tensor_scalar(
                                mn[:], h_ps[:], 0.0, float(prelu_slope),
                                mybir.AluOpType.min, mybir.AluOpType.mult)
                            nc.vector.tensor_scalar_max(mx[:], h_ps[:], 0.0)
                            nc.vector.tensor_tensor(
                                out=o_t[:], in0=mn[:], in1=mx[:],
                                op=mybir.AluOpType.add)

                        rb = min(P, T - b * P)
                        if l == 2:
                            nc.sync.dma_start(
                                out=y[b * P:b * P + rb, :], in_=o_t[:rb, :])
                        elif b < ablk:
                            nc.sync.dma_start(
                                out=bncA[l][b * P:b * P + rb, :],
                                in_=o_t[:rb, :])
                        else:
                            r0 = (b - ablk) * P
                            nc.sync.dma_start(
                                out=bncB[l][r0:r0 + rb, :], in_=o_t[:rb, :])

                    if l < 2 and g == na_groups - 1:
                        nc.gpsimd.collective_compute(
                            "AllGather", mybir.AluOpType.bypass,
                            replica_groups=[list(range(NCORES))],
                            ins=[bncA[l].opt()], outs=[tblsA[l].opt()])
                if l < 2:
                    nc.gpsimd.collective_compute(
                        "AllGather", mybir.AluOpType.bypass,
                        replica_groups=[list(range(NCORES))],
                        ins=[bncB[l].opt()], outs=[tblsB[l].opt()])

    nc.compile()
    return nc


_CACHE = {}


def kernel(x, edge_index, edge_weights, W1, b1, W2, b2, W3, b3, prelu_w):
    """Full (unsharded) inputs in, full [50000, 128] fp32 output out."""
    st = _make_structure(N_NODES, group_size=2)
    idx_arr, S_arr, nrm_arr, meta = _preprocess(edge_index, edge_weights, st)

    key = (meta["NSLOT"], meta["STOT"], float(prelu_w))
    if key not in _CACHE:
        _CACHE[key] = _build_kernel(st, meta, float(prelu_w))
    nc = _CACHE[key]

    T, arows = st["T"], st["arows"]
    x16 = np.asarray(x, dtype=np.float16).reshape(NCORES, T, D)
    xa = np.ascontiguousarray(x16[:, :arows, :].reshape(-1, D))
    xb = np.ascontiguousarray(x16[:, arows:, :].reshape(-1, D))
    wts = np.ascontiguousarray(np.concatenate(
        [np.asarray(W, dtype=np.float16) for W in (W1, W2, W3)], axis=1))
    bias = np.ascontiguousarray(np.concatenate(
        [np.asarray(b, dtype=np.float32) for b in (b1, b2, b3)])[None, :])
    in_maps = [
        {"xa": xa, "xb": xb, "s": np.ascontiguousarray(S_arr[c]),
         "idx": np.ascontiguousarray(idx_arr[c]), "wts": wts, "bias": bias,
         "nrm": np.ascontiguousarray(nrm_arr[c])}
        for c in range(NCORES)
    ]

    res = run_bass_kernel_spmd(nc, in_maps, core_ids=list(range(NCORES)))
    out = np.concatenate([res.results[c]["y"] for c in range(NCORES)], axis=0)
    return out.astype(np.float32)
